# revision 6
# baseline (speedup 1.0000x reference)
"""Trainium2 Bass kernel for nn_PiecewiseNDModel (histogram_binning).

out[n] = values[i0[n], i1[n], i2[n]] where i_d = clip(count(x_d >= bp_d) - 1, 0, 63).

Sharding (per hint): batch split across 8 NeuronCores (1,048,576 elements each);
breakpoints + 64^3 table replicated per core.

Per core:
  1. Bucketize on VectorE: c_d = sum_{j>=1}(x_d >= bp_d[j]) over bp[1..63]
     (skipping bp[0] makes clip() free).  Breakpoints are baked as f32
     immediates into fused scalar_tensor_tensor compare-accumulate ops.
     flat = (c0*64 + c1)*64 + c2  (exact in f32, < 2^24).
  2. Split flat = q*16384 + r.  Partition P of SBUF holds table chunk
     (P mod 16) (16384 f32 = 64KB).  One ap_gather (GPSIMD) with idx=r
     fetches, for each element, the 16 chunk-candidates across its
     Q7-core's 16 partitions.  A second ap_gather with idx=q into a
     16x16 identity table yields the selection mask M directly.
  3. MR = M*R on VectorE; PE matmul with a block-indicator weight sums
     each 16-partition group -> the selected value, exactly one nonzero
     term per sum.
"""

import sys

sys.path.insert(0, "/opt/trn_rl_repo")

import numpy as np

import concourse.bass as bass
import concourse.bacc as bacc
import concourse.mybir as mybir
import concourse.tile as tile
from concourse.bass_utils import run_bass_kernel_spmd

BATCH = 8388608
NBP = 64
NCORES = 8
PER_CORE = BATCH // NCORES          # 1048576
P = 128
F = PER_CORE // P                   # 8192 home columns per core
FC = 512                            # home columns per chunk
NCHUNK = F // FC
SG = 16 * FC                        # stream (gather) positions per Q7-core per chunk
NMM = SG // 512                     # matmuls per chunk
CHUNK_ELEMS = 16384                 # table elements per partition (q in [0,16))

_PROGRAM_CACHE = {}


def _build_program(bp0, bp1, bp2):
    key = (bp0.tobytes(), bp1.tobytes(), bp2.tobytes())
    if key in _PROGRAM_CACHE:
        return _PROGRAM_CACHE[key]

    f32 = mybir.dt.float32
    nc = bacc.Bacc("TRN2", target_bir_lowering=False, debug=False)
    x_in = [nc.dram_tensor(f"x{d}", [P, F], f32, kind="ExternalInput")
            for d in range(3)]
    vchunks_in = nc.dram_tensor("vchunks", [P, CHUNK_ELEMS], f32,
                                kind="ExternalInput")
    ident_in = nc.dram_tensor("ident16", [P, 16], f32, kind="ExternalInput")
    wred_in = nc.dram_tensor("wred", [P, 8], f32, kind="ExternalInput")
    out_ext = nc.dram_tensor("out", [8, NCHUNK * SG], f32,
                             kind="ExternalOutput")

    bps = [np.asarray(b, dtype=np.float32) for b in (bp0, bp1, bp2)]
    ge = mybir.AluOpType.is_ge
    add = mybir.AluOpType.add
    mult = mybir.AluOpType.mult

    with tile.TileContext(nc) as tc:
        with (
            tc.tile_pool(name="const", bufs=1) as cpool,
            tc.tile_pool(name="xs", bufs=2) as xpool,
            tc.tile_pool(name="work", bufs=1) as wpool,
            tc.tile_pool(name="psum", bufs=2, space="PSUM") as ppool,
        ):
            vtab = cpool.tile([P, CHUNK_ELEMS], f32)
            nc.sync.dma_start(vtab[:], vchunks_in.ap())
            ident = cpool.tile([P, 16], f32)
            nc.sync.dma_start(ident[:], ident_in.ap())
            wred = cpool.tile([P, 8], f32)
            nc.sync.dma_start(wred[:], wred_in.ap())

            for ci in range(NCHUNK):
                sl = bass.ts(ci, FC)
                xt = []
                for d in range(3):
                    t = xpool.tile([P, FC], f32, tag=f"x{d}")
                    nc.sync.dma_start(t[:], x_in[d].ap()[:, sl])
                    xt.append(t)

                # --- bucketize: acc = ((c0*64)+c1)*64+c2 ---
                acc = wpool.tile([P, FC], f32, tag="acc")
                for d in range(3):
                    bp = bps[d]
                    if d == 0:
                        nc.vector.tensor_scalar(acc[:], xt[0][:],
                                                float(bp[1]), None, ge)
                    else:
                        nc.vector.tensor_scalar(acc[:], acc[:], 64.0, None, mult)
                        nc.vector.scalar_tensor_tensor(
                            acc[:], xt[d][:], float(bp[1]), acc[:], ge, add)
                    for j in range(2, NBP):
                        nc.vector.scalar_tensor_tensor(
                            acc[:], xt[d][:], float(bp[j]), acc[:], ge, add)

                # --- flat -> (r, q) int16 ---
                flat_i = wpool.tile([P, FC], mybir.dt.int32, tag="flat_i")
                nc.vector.tensor_copy(flat_i[:], acc[:])
                q_i = wpool.tile([P, FC], mybir.dt.int32, tag="q_i")
                nc.vector.tensor_scalar(q_i[:], flat_i[:], 14,
                                        None, mybir.AluOpType.logical_shift_right)
                q16 = wpool.tile([P, FC], mybir.dt.int16, tag="q16")
                nc.vector.tensor_copy(q16[:], q_i[:])
                nc.vector.tensor_scalar(flat_i[:], flat_i[:], CHUNK_ELEMS - 1,
                                        None, mybir.AluOpType.bitwise_and)
                r16 = wpool.tile([P, FC], mybir.dt.int16, tag="r16")
                nc.vector.tensor_copy(r16[:], flat_i[:])

                # --- gathers (GPSIMD, per-Q7-core 16-wrapped streams) ---
                rt = wpool.tile([P, SG], f32, tag="rt")
                nc.gpsimd.ap_gather(rt[:], vtab[:], r16[:], channels=P,
                                    num_elems=CHUNK_ELEMS, d=1, num_idxs=SG)
                mt = wpool.tile([P, SG], f32, tag="mt")
                nc.gpsimd.ap_gather(mt[:], ident[:], q16[:], channels=P,
                                    num_elems=16, d=1, num_idxs=SG)

                # --- select + 16->1 reduce ---
                nc.vector.tensor_tensor(rt[:], rt[:], mt[:], mult)
                for mi in range(NMM):
                    ms = bass.ts(mi, 512)
                    pt = ppool.tile([8, 512], f32, tag="pt")
                    nc.tensor.matmul(out=pt[:], lhsT=wred[:, :8],
                                     rhs=rt[:, ms], start=True, stop=True)
                    ot = wpool.tile([8, 512], f32, tag="ot")
                    nc.scalar.copy(ot[:], pt[:])
                    nc.sync.dma_start(
                        out_ext.ap()[:, bass.ts(ci * NMM + mi, 512)], ot[:])
    nc.finalize()
    _PROGRAM_CACHE[key] = nc
    return nc


def _host_constants(values):
    v = np.ascontiguousarray(np.asarray(values, dtype=np.float32).reshape(-1))
    # partition P holds chunk (P % 16)
    vchunks = np.empty((P, CHUNK_ELEMS), dtype=np.float32)
    for p in range(P):
        c = p % 16
        vchunks[p] = v[c * CHUNK_ELEMS:(c + 1) * CHUNK_ELEMS]
    ident = np.zeros((P, 16), dtype=np.float32)
    for p in range(P):
        ident[p, p % 16] = 1.0
    wred = np.zeros((P, 8), dtype=np.float32)
    for p in range(P):
        wred[p, p // 16] = 1.0
    return vchunks, ident, wred


def kernel(x0, x1, x2, bp0, bp1, bp2, values):
    nc = _build_program(np.asarray(bp0), np.asarray(bp1), np.asarray(bp2))
    vchunks, ident, wred = _host_constants(values)
    in_maps = []
    for c in range(NCORES):
        sh = slice(c * PER_CORE, (c + 1) * PER_CORE)
        in_maps.append({
            "x0": np.ascontiguousarray(np.asarray(x0[sh]).reshape(P, F)),
            "x1": np.ascontiguousarray(np.asarray(x1[sh]).reshape(P, F)),
            "x2": np.ascontiguousarray(np.asarray(x2[sh]).reshape(P, F)),
            "vchunks": vchunks,
            "ident16": ident,
            "wred": wred,
        })
    res = run_bass_kernel_spmd(nc, in_maps, core_ids=list(range(NCORES)))
    out = np.empty(BATCH, dtype=np.float32)
    for c in range(NCORES):
        oc = res.results[c]["out"]              # [8, NCHUNK*SG]
        oc = oc.reshape(8, NCHUNK, FC, 16)      # [q, ci, s, p]
        home = oc.transpose(0, 3, 1, 2)         # [q, p, ci, s]
        out[c * PER_CORE:(c + 1) * PER_CORE] = home.reshape(-1)
    return out


# revision 9
# speedup vs baseline: 1.0796x; 1.0796x over previous
"""Trainium2 Bass kernel for nn_PiecewiseNDModel (histogram_binning).

out[n] = values[i0[n], i1[n], i2[n]] where i_d = clip(count(x_d >= bp_d) - 1, 0, 63).

Sharding (per hint): batch split across 8 NeuronCores (1,048,576 elements each);
breakpoints + 64^3 table replicated per core.

Per core:
  1. Bucketize on VectorE: c_d = sum_{j>=1}(x_d >= bp_d[j]) over bp[1..63]
     (skipping bp[0] makes clip() free).  Breakpoints are baked as f32
     immediates into fused scalar_tensor_tensor compare-accumulate ops.
     flat = (c0*64 + c1)*64 + c2  (exact in f32, < 2^24).
  2. Split flat = q*16384 + r.  Partition P of SBUF holds table chunk
     (P mod 16) (16384 f32 = 64KB).  One ap_gather (GPSIMD) with idx=r
     fetches, for each element, the 16 chunk-candidates across its
     Q7-core's 16 partitions.  A second ap_gather with idx=q into a
     16x16 identity table yields the selection mask M directly.
  3. MR = M*R on VectorE; PE matmul with a block-indicator weight sums
     each 16-partition group -> the selected value; exactly one nonzero
     term per sum, so the fp32 matmul is exact.

Element bookkeeping: core c takes elements [c*2^20, (c+1)*2^20), reshaped
C-order to the [128, 8192] home layout.  ap_gather consumes indices wrapped
16-way per Q7-core (stream position n = s*16 + p), which matches the home
layout directly, and the PE reduce emits [8, stream] tiles; the host inverts
that permutation for free.
"""

import sys

sys.path.insert(0, "/opt/trn_rl_repo")

import numpy as np

import concourse.bass as bass
import concourse.bacc as bacc
import concourse.mybir as mybir
import concourse.tile as tile
from concourse.bass_utils import run_bass_kernel_spmd

BATCH = 8388608
NBP = 64
NCORES = 8
PER_CORE = BATCH // NCORES          # 1048576
P = 128
F = PER_CORE // P                   # 8192 home columns per core
FC = 512                            # home columns per chunk
NCHUNK = F // FC
SG = 16 * FC                        # stream (gather) positions per Q7-core per chunk
NMM = SG // 512                     # matmuls per chunk
CHUNK_ELEMS = 16384                 # table elements per partition (q in [0,16))

_PROGRAM_CACHE = {}


def _build_program(bp0, bp1, bp2):
    key = (bp0.tobytes(), bp1.tobytes(), bp2.tobytes())
    if key in _PROGRAM_CACHE:
        return _PROGRAM_CACHE[key]

    f32 = mybir.dt.float32
    nc = bacc.Bacc("TRN2", target_bir_lowering=False, debug=False)
    x_in = [nc.dram_tensor(f"x{d}", [P, F], f32, kind="ExternalInput")
            for d in range(3)]
    vchunks_in = nc.dram_tensor("vchunks", [P, CHUNK_ELEMS], f32,
                                kind="ExternalInput")
    ident_in = nc.dram_tensor("ident16", [P, 16], f32, kind="ExternalInput")
    wred_in = nc.dram_tensor("wred", [P, 8], f32, kind="ExternalInput")
    out_ext = nc.dram_tensor("out", [8, NCHUNK * SG], f32,
                             kind="ExternalOutput")

    bps = [np.asarray(b, dtype=np.float32) for b in (bp0, bp1, bp2)]
    ge = mybir.AluOpType.is_ge
    add = mybir.AluOpType.add
    mult = mybir.AluOpType.mult

    with tile.TileContext(nc) as tc:
        with (
            tc.tile_pool(name="const", bufs=1) as cpool,
            tc.tile_pool(name="xs", bufs=2) as xpool,
            tc.tile_pool(name="work", bufs=1) as wpool,
            tc.tile_pool(name="psum", bufs=2, space="PSUM") as ppool,
        ):
            vtab = cpool.tile([P, CHUNK_ELEMS], f32)
            nc.sync.dma_start(vtab[:], vchunks_in.ap())
            ident = cpool.tile([P, 16], f32)
            nc.sync.dma_start(ident[:], ident_in.ap())
            wred = cpool.tile([P, 8], f32)
            nc.sync.dma_start(wred[:], wred_in.ap())

            for ci in range(NCHUNK):
                sl = bass.ts(ci, FC)
                xt = []
                for d in range(3):
                    t = xpool.tile([P, FC], f32, tag=f"x{d}")
                    nc.sync.dma_start(t[:], x_in[d].ap()[:, sl])
                    xt.append(t)

                # --- bucketize: acc = ((c0*64)+c1)*64+c2 ---
                acc = wpool.tile([P, FC], f32, tag="acc")
                for d in range(3):
                    bp = bps[d]
                    if d == 0:
                        nc.vector.tensor_scalar(acc[:], xt[0][:],
                                                float(bp[1]), None, ge)
                    else:
                        nc.vector.tensor_scalar(acc[:], acc[:], 64.0, None, mult)
                        nc.vector.scalar_tensor_tensor(
                            acc[:], xt[d][:], float(bp[1]), acc[:], ge, add)
                    for j in range(2, NBP):
                        nc.vector.scalar_tensor_tensor(
                            acc[:], xt[d][:], float(bp[j]), acc[:], ge, add)

                # --- flat -> (r, q) int16 ---
                flat_i = wpool.tile([P, FC], mybir.dt.int32, tag="flat_i")
                nc.vector.tensor_copy(flat_i[:], acc[:])
                q_i = wpool.tile([P, FC], mybir.dt.int32, tag="q_i")
                nc.vector.tensor_scalar(q_i[:], flat_i[:], 14,
                                        None, mybir.AluOpType.logical_shift_right)
                q16 = wpool.tile([P, FC], mybir.dt.int16, tag="q16")
                nc.vector.tensor_copy(q16[:], q_i[:])
                nc.vector.tensor_scalar(flat_i[:], flat_i[:], CHUNK_ELEMS - 1,
                                        None, mybir.AluOpType.bitwise_and)
                r16 = wpool.tile([P, FC], mybir.dt.int16, tag="r16")
                nc.vector.tensor_copy(r16[:], flat_i[:])

                # --- gathers (GPSIMD, per-Q7-core 16-wrapped streams) ---
                rt = wpool.tile([P, SG], f32, tag="rt")
                nc.gpsimd.ap_gather(rt[:], vtab[:], r16[:], channels=P,
                                    num_elems=CHUNK_ELEMS, d=1, num_idxs=SG)
                mt = wpool.tile([P, SG], f32, tag="mt")
                nc.gpsimd.ap_gather(mt[:], ident[:], q16[:], channels=P,
                                    num_elems=16, d=1, num_idxs=SG)

                # --- select + 16->1 reduce ---
                nc.vector.tensor_tensor(rt[:], rt[:], mt[:], mult)
                for mi in range(NMM):
                    ms = bass.ts(mi, 512)
                    pt = ppool.tile([8, 512], f32, tag="pt")
                    nc.tensor.matmul(out=pt[:], lhsT=wred[:, :8],
                                     rhs=rt[:, ms], start=True, stop=True)
                    ot = wpool.tile([8, 512], f32, tag="ot")
                    nc.scalar.copy(ot[:], pt[:])
                    nc.sync.dma_start(
                        out_ext.ap()[:, bass.ts(ci * NMM + mi, 512)], ot[:])
    nc.finalize()
    _PROGRAM_CACHE[key] = nc
    return nc


def _host_constants(values):
    v = np.ascontiguousarray(np.asarray(values, dtype=np.float32).reshape(-1))
    # partition P holds chunk (P % 16)
    vchunks = np.empty((P, CHUNK_ELEMS), dtype=np.float32)
    for p in range(P):
        c = p % 16
        vchunks[p] = v[c * CHUNK_ELEMS:(c + 1) * CHUNK_ELEMS]
    ident = np.zeros((P, 16), dtype=np.float32)
    for p in range(P):
        ident[p, p % 16] = 1.0
    wred = np.zeros((P, 8), dtype=np.float32)
    for p in range(P):
        wred[p, p // 16] = 1.0
    return vchunks, ident, wred


def kernel(x0, x1, x2, bp0, bp1, bp2, values):
    nc = _build_program(np.asarray(bp0), np.asarray(bp1), np.asarray(bp2))
    vchunks, ident, wred = _host_constants(values)
    in_maps = []
    for c in range(NCORES):
        sh = slice(c * PER_CORE, (c + 1) * PER_CORE)
        in_maps.append({
            "x0": np.ascontiguousarray(np.asarray(x0[sh]).reshape(P, F)),
            "x1": np.ascontiguousarray(np.asarray(x1[sh]).reshape(P, F)),
            "x2": np.ascontiguousarray(np.asarray(x2[sh]).reshape(P, F)),
            "vchunks": vchunks,
            "ident16": ident,
            "wred": wred,
        })
    res = run_bass_kernel_spmd(nc, in_maps, core_ids=list(range(NCORES)))
    out = np.empty(BATCH, dtype=np.float32)
    for c in range(NCORES):
        oc = res.results[c]["out"]              # [8, NCHUNK*SG]
        oc = oc.reshape(8, NCHUNK, FC, 16)      # [q, ci, s, p]
        home = oc.transpose(0, 3, 1, 2)         # [q, p, ci, s]
        out[c * PER_CORE:(c + 1) * PER_CORE] = home.reshape(-1)
    return out
